# revision 2
# baseline (speedup 1.0000x reference)
"""Trainium2 Bass kernel for the 2-layer LSTM language model (v2).

Strategy: 8-way tensor parallelism over the hidden/gate dimension, with the
per-step h exchange done via remote_dma_broadcast (direct SBUF->SBUF SDMA to
all 8 cores on the chip) instead of the ncfw collective_compute AllGather.
The collective costs ~20-70us of control-plane latency per call inside the
sequential recurrence; the RDMA broadcast is ~1-2us and overlaps with compute.

- Core k owns hidden slice k (128 of 1024 units) of both LSTM layers and
  computes gate columns [f_k|i_k|o_k|t_k] (512 of 4096) each step.
- Per step each core broadcasts bc1 = h0T_k(t) and bc2 = (h1T_k(t-1) |
  hsT_k(t-2)) from SBUF into every core's SBUF receive ring (NB slots).
  Layer 1 lags layer 0 by one step; the output-MLP hidden lags by two.
- Arrival tracking is per-sender (16 semaphores); a gpsimd relay collapses
  the 8-way waits into one s_rdy increment for the tensor engine. Per-sender
  per-lane semaphore updates are delivered in order behind their data, which
  makes the wait thresholds exact (no cross-sender counting).
- The output MLP is streamed inside the time loop, column-sharded: core k
  computes hs columns [k*128,(k+1)*128) from the gathered h1, broadcasts the
  transposed slice, then computes logits columns [k*32,(k+1)*32) from the
  gathered hsT. Final logits are column-concatenated on the host.
- The embedding, the speaker-flag rank-1 term, and b0 are folded into a
  single [384, 4096] input-side weight on the host.
Matmul operands are bf16 (f32 PSUM accumulation); cell state stays f32.
"""
import numpy as np
import ml_dtypes

import concourse.bass as bass
import concourse.mybir as mybir
from concourse.bass_utils import run_bass_kernel_spmd

BF16 = ml_dtypes.bfloat16

T_FULL, B, IND = 512, 128, 259
EMB, NN, VOCAB, BIG = 512, 1024, 256, 128
NC = 8
SL = NN // NC          # 128 hidden units per core
GC = 4 * SL            # 512 gate columns per core
KP = 384               # padded inpT rows = 3 K-tiles (259 data + 1 + s + pad)
OSL = VOCAB // NC      # 32 logit columns per core
NB = 8                 # receive ring depth (slots)
SLOT = 3 * 128         # cols per sender per slot: h0T | h1T | hsT
AF = mybir.dt.ActivationFunctionType if hasattr(mybir.dt, "ActivationFunctionType") else mybir.ActivationFunctionType
BF = mybir.dt.bfloat16
F32 = mybir.dt.float32


def build(T):
    nc = bass.Bass(target_bir_lowering=False, num_devices=NC,
                   detect_race_conditions=False)

    # ---- DRAM parameters (per core) ----
    inpT = nc.declare_dram_parameter("inpT", [KP, T * B], BF, isOutput=False)
    wc = nc.declare_dram_parameter("wc", [KP, GC], BF, isOutput=False)
    w0h = nc.declare_dram_parameter("w0h", [NN, GC], BF, isOutput=False)
    w1x = nc.declare_dram_parameter("w1x", [NN, GC], BF, isOutput=False)
    w1h = nc.declare_dram_parameter("w1h", [NN, GC], BF, isOutput=False)
    b1r = nc.declare_dram_parameter("b1r", [1, GC], BF, isOutput=False)
    ow0k = nc.declare_dram_parameter("ow0k", [NN, SL], BF, isOutput=False)
    ob0k = nc.declare_dram_parameter("ob0k", [128, 1], F32, isOutput=False)
    ow1k = nc.declare_dram_parameter("ow1k", [NN, OSL], BF, isOutput=False)
    ob1k = nc.declare_dram_parameter("ob1k", [1, OSL], BF, isOutput=False)
    iden = nc.declare_dram_parameter("iden", [128, 128], BF, isOutput=False)
    out = nc.declare_dram_parameter("out", [B, T * OSL], F32, isOutput=True)

    # ---- SBUF ----
    wc_sb = nc.alloc_sbuf_tensor("wc_sb", [128, 3 * GC], BF)
    w0h_sb = nc.alloc_sbuf_tensor("w0h_sb", [128, 8 * GC], BF)
    w1x_sb = nc.alloc_sbuf_tensor("w1x_sb", [128, 8 * GC], BF)
    w1h_sb = nc.alloc_sbuf_tensor("w1h_sb", [128, 8 * GC], BF)
    b1_sb = nc.alloc_sbuf_tensor("b1_sb", [1, GC], BF)
    ow0_sb = nc.alloc_sbuf_tensor("ow0_sb", [128, 8 * SL], BF)
    ow1_sb = nc.alloc_sbuf_tensor("ow1_sb", [128, 8 * OSL], BF)
    ob0_sb = nc.alloc_sbuf_tensor("ob0_sb", [128, 1], F32)
    ob1_sb = nc.alloc_sbuf_tensor("ob1_sb", [1, OSL], BF)
    ones_sb = nc.alloc_sbuf_tensor("ones_sb", [1, 128], BF)
    id_sb = nc.alloc_sbuf_tensor("id_sb", [128, 128], BF)
    xb_sb = nc.alloc_sbuf_tensor("xb_sb", [128, 4 * 3 * 128], BF)  # 4 bufs x 3 tiles
    hT_sb = nc.alloc_sbuf_tensor("hT_sb", [128, NB * NC * SLOT], BF)  # receive ring
    g0_sb = nc.alloc_sbuf_tensor("g0_sb", [128, 3 * SL], F32)
    t0_sb = nc.alloc_sbuf_tensor("t0_sb", [128, SL], F32)
    th0_sb = nc.alloc_sbuf_tensor("th0_sb", [128, SL], F32)
    g1_sb = nc.alloc_sbuf_tensor("g1_sb", [128, 3 * SL], F32)
    t1_sb = nc.alloc_sbuf_tensor("t1_sb", [128, SL], F32)
    th1_sb = nc.alloc_sbuf_tensor("th1_sb", [128, SL], F32)
    c0_sb = nc.alloc_sbuf_tensor("c0_sb", [128, SL], F32)
    c1_sb = nc.alloc_sbuf_tensor("c1_sb", [128, SL], F32)
    tmpa_sb = nc.alloc_sbuf_tensor("tmpa_sb", [128, SL], F32)
    tmpb_sb = nc.alloc_sbuf_tensor("tmpb_sb", [128, SL], F32)
    h0l_sb = nc.alloc_sbuf_tensor("h0l_sb", [128, SL], BF)
    h1l_sb = nc.alloc_sbuf_tensor("h1l_sb", [128, SL], BF)
    send_sb = nc.alloc_sbuf_tensor("send_sb", [128, 2 * SLOT], BF)  # 2 bufs
    lg_sb = nc.alloc_sbuf_tensor("lg_sb", [128, 32 * OSL], F32)  # 32-slot logit ring

    # ---- PSUM ----
    psA = nc.alloc_psum_tensor("psA", [128, 512], F32)
    psB = nc.alloc_psum_tensor("psB", [128, 512], F32)
    psT0 = nc.alloc_psum_tensor("psT0", [128, 128], BF)
    psT1 = nc.alloc_psum_tensor("psT1", [128, 128], BF)
    psH = nc.alloc_psum_tensor("psH", [128, 128], F32)
    psL = nc.alloc_psum_tensor("psL", [128, OSL], F32)

    # ---- semaphores ----
    sems = {}
    for name in (
        "s_init", "s_vinit", "s_x", "s_xdone", "s_z0", "s_z1", "s_act0",
        "s_act1", "s_cmid0", "s_cmid1", "s_th0", "s_th1", "s_dve0", "s_dve1",
        "s_t0", "s_t1", "s_cp0", "s_cp1", "s_hs", "s_hsT", "s_lg", "s_lgcp",
        "s_out", "s_prep", "s_snt1", "s_snt2", "s_rdy1", "s_rdy2",
    ):
        sems[name] = nc.alloc_semaphore(name)
    s_a1 = [nc.alloc_semaphore(f"s_a1_{k}") for k in range(NC)]
    s_a2 = [nc.alloc_semaphore(f"s_a2_{k}") for k in range(NC)]

    N_INIT = 47

    # send schedule: bc1(t) for t in [0, T-1] carries h0T(t);
    # bc2(t) for t in [0, T+1] carries (h1T(t-1) | hsT(t-2)).
    def has_bc1(t):
        return t <= T - 1

    def has_bc2(t):
        return t <= T + 1

    # send-buffer release threshold: all sends of iters <= tau completed
    def SNT1(tau):
        return 16 * sum(1 for u in range(tau + 1) if has_bc1(u))

    def SNT2(tau):
        return 16 * sum(1 for u in range(tau + 1) if has_bc2(u))

    def slot_off(t):
        return (t % NB) * NC * SLOT

    TEND = T + 2  # iters 0..T+2 inclusive

    with nc.Block() as block:

        @block.sync
        def _(sync):
            n_init = 0

            def ld(dst, src):
                nonlocal n_init
                sync.dma_start(out=dst, in_=src).then_inc(sems["s_init"], 16)
                n_init += 1

            for j in range(3):
                ld(wc_sb[:, j * GC:(j + 1) * GC], wc[j * 128:(j + 1) * 128, :])
            for j in range(8):
                ld(w0h_sb[:, j * GC:(j + 1) * GC], w0h[j * 128:(j + 1) * 128, :])
                ld(w1x_sb[:, j * GC:(j + 1) * GC], w1x[j * 128:(j + 1) * 128, :])
                ld(w1h_sb[:, j * GC:(j + 1) * GC], w1h[j * 128:(j + 1) * 128, :])
            for j in range(8):
                ld(ow0_sb[:, j * SL:(j + 1) * SL], ow0k[j * 128:(j + 1) * 128, :])
                ld(ow1_sb[:, j * OSL:(j + 1) * OSL], ow1k[j * 128:(j + 1) * 128, :])
            ld(b1_sb[:, :], b1r[:, :])
            ld(id_sb[:, :], iden[:, :])
            ld(ob0_sb[:, :], ob0k[:, :])
            ld(ob1_sb[:, :], ob1k[:, :])
            assert n_init == N_INIT, n_init
            # x prefetch: inpT column block t -> xb slot t%4 (3 k-tiles)
            inpT3 = inpT.rearrange("(j p) n -> p j n", p=128)
            for t in range(T):
                if t >= 1:
                    sync.wait_ge(sems["s_x"], 16 * t)
                if t >= 4:
                    sync.wait_ge(sems["s_xdone"], t - 3)
                s = (t % 4) * 3 * 128
                dst = xb_sb[:, s:s + 3 * 128].rearrange("p (j c) -> p j c", c=128)
                sync.dma_start(
                    out=dst, in_=inpT3[:, :, t * B:(t + 1) * B]
                ).then_inc(sems["s_x"], 16)

        @block.tensor
        def _(tensor):
            tensor.wait_ge(sems["s_init"], 16 * N_INIT)
            tensor.wait_ge(sems["s_vinit"], 1)
            for t in range(TEND + 1):
                so = slot_off(t - 1)  # all iter-t reads hit slot t-1
                if t <= T - 1:
                    # z0(t) into psA
                    tensor.wait_ge(sems["s_x"], 16 * (t + 1))
                    if t >= 1:
                        tensor.wait_ge(sems["s_act0"], t)  # psA WAR
                    xoff = (t % 4) * 3 * 128
                    for j in range(3):
                        ins = tensor.matmul(
                            psA[:, :],
                            xb_sb[:, xoff + j * 128:xoff + (j + 1) * 128],
                            wc_sb[:, j * GC:(j + 1) * GC],
                            start=(j == 0),
                            stop=(t == 0 and j == 2),
                        )
                        if j == 2:
                            ins.then_inc(sems["s_xdone"], 1)
                    if t >= 1:
                        tensor.wait_ge(sems["s_rdy1"], t)  # bc1(t-1) arrived
                        for j in range(8):
                            ins = tensor.matmul(
                                psA[:, :],
                                hT_sb[:, so + j * SLOT:so + j * SLOT + 128],
                                w0h_sb[:, j * GC:(j + 1) * GC],
                                start=False,
                                stop=(j == 7),
                            )
                            if j == 7:
                                ins.then_inc(sems["s_z0"], 1)
                if 1 <= t <= T:
                    # z1(t-1) into psB
                    if t >= 2:
                        tensor.wait_ge(sems["s_act1"], t - 1)  # psB WAR
                    tensor.wait_ge(sems["s_rdy2"], t)  # bc2(t-1) arrived
                    tensor.matmul(psB[:, :], ones_sb[:, :], b1_sb[:, :], start=True, stop=False)
                    for j in range(8):
                        tensor.matmul(
                            psB[:, :],
                            hT_sb[:, so + j * SLOT:so + j * SLOT + 128],
                            w1x_sb[:, j * GC:(j + 1) * GC],
                            start=False, stop=False,
                        )
                    for j in range(8):
                        ins = tensor.matmul(
                            psB[:, :],
                            hT_sb[:, so + j * SLOT + 128:so + j * SLOT + 256],
                            w1h_sb[:, j * GC:(j + 1) * GC],
                            start=False, stop=(j == 7),
                        )
                        if j == 7:
                            ins.then_inc(sems["s_z1"], 1)
                if 2 <= t <= T + 1:
                    # hs(t-2) slice k into psH (hsT layout: hs-cols x batch)
                    if t == T + 1:
                        tensor.wait_ge(sems["s_rdy2"], t)
                    if t >= 3:
                        tensor.wait_ge(sems["s_hsT"], t - 2)  # psH WAR
                    for j in range(8):
                        ins = tensor.matmul(
                            psH[:, :],
                            ow0_sb[:, j * SL:(j + 1) * SL],
                            hT_sb[:, so + j * SLOT + 128:so + j * SLOT + 256],
                            start=(j == 0), stop=(j == 7),
                        )
                        if j == 7:
                            ins.then_inc(sems["s_hs"], 1)
                if t <= T - 1:
                    # transpose h0(t)
                    tensor.wait_ge(sems["s_dve0"], t + 1)
                    if t >= 1:
                        tensor.wait_ge(sems["s_cp0"], t)  # psT0 WAR
                    tensor.transpose(psT0[:, 0:128], h0l_sb[:, :], id_sb[:, :]).then_inc(
                        sems["s_t0"], 1
                    )
                if 1 <= t <= T:
                    # transpose h1(t-1)
                    tensor.wait_ge(sems["s_dve1"], t)
                    if t >= 2:
                        tensor.wait_ge(sems["s_cp1"], t - 1)  # psT1 WAR
                    tensor.transpose(psT1[:, 0:128], h1l_sb[:, :], id_sb[:, :]).then_inc(
                        sems["s_t1"], 1
                    )
                if 3 <= t <= T + 2:
                    # logits(t-3) slice k into psL
                    if t == T + 2:
                        tensor.wait_ge(sems["s_rdy2"], t)
                    if t >= 4:
                        tensor.wait_ge(sems["s_lgcp"], t - 3)  # psL WAR
                    tensor.matmul(psL[:, :], ones_sb[:, :], ob1_sb[:, :], start=True, stop=False)
                    for j in range(8):
                        ins = tensor.matmul(
                            psL[:, :],
                            hT_sb[:, so + j * SLOT + 256:so + j * SLOT + 384],
                            ow1_sb[:, j * OSL:(j + 1) * OSL],
                            start=False, stop=(j == 7),
                        )
                        if j == 7:
                            ins.then_inc(sems["s_lg"], 1)

        @block.scalar
        def _(scalar):
            scalar.wait_ge(sems["s_init"], 16 * N_INIT)
            SIG = AF.Sigmoid
            TANH = AF.Tanh
            for t in range(TEND + 1):
                if t <= T - 1:
                    if t == 0:
                        scalar.wait_ge(sems["s_xdone"], 1)
                    else:
                        scalar.wait_ge(sems["s_z0"], t)
                    if t >= 1:
                        scalar.wait_ge(sems["s_cmid0"], t)  # g0/t0 WAR
                    scalar.activation(g0_sb[:, :], psA[:, 0:3 * SL], SIG)
                    scalar.activation(t0_sb[:, :], psA[:, 3 * SL:4 * SL], TANH).then_inc(
                        sems["s_act0"], 1
                    )
                    scalar.wait_ge(sems["s_cmid0"], t + 1)
                    if t >= 1:
                        scalar.wait_ge(sems["s_dve0"], t)  # th0 WAR
                    scalar.activation(th0_sb[:, :], c0_sb[:, :], TANH).then_inc(
                        sems["s_th0"], 1
                    )
                if 1 <= t <= T:
                    scalar.wait_ge(sems["s_z1"], t)
                    if t >= 2:
                        scalar.wait_ge(sems["s_cmid1"], t - 1)
                    scalar.activation(g1_sb[:, :], psB[:, 0:3 * SL], SIG)
                    scalar.activation(t1_sb[:, :], psB[:, 3 * SL:4 * SL], TANH).then_inc(
                        sems["s_act1"], 1
                    )
                    scalar.wait_ge(sems["s_cmid1"], t)
                    if t >= 2:
                        scalar.wait_ge(sems["s_dve1"], t - 1)
                    scalar.activation(th1_sb[:, :], c1_sb[:, :], TANH).then_inc(
                        sems["s_th1"], 1
                    )
                if 2 <= t <= T + 1:
                    # relu(hs(t-2)) + bias -> send buf hsT block (bf16)
                    scalar.wait_ge(sems["s_hs"], t - 1)
                    scalar.wait_ge(sems["s_snt1"], SNT1(t - 2))
                    scalar.wait_ge(sems["s_snt2"], SNT2(t - 2))
                    sb = (t % 2) * SLOT
                    scalar.activation(
                        send_sb[:, sb + 256:sb + 384],
                        psH[:, :],
                        AF.Relu,
                        bias=ob0_sb[:, 0:1],
                    ).then_inc(sems["s_hsT"], 1)

        @block.vector
        def _(vector):
            vector.memset(send_sb[:, :], 0.0)
            vector.memset(c0_sb[:, :], 0.0)
            vector.memset(c1_sb[:, :], 0.0)
            vector.memset(ones_sb[:, :], 1.0).then_inc(sems["s_vinit"], 1)
            MUL = mybir.AluOpType.mult
            for t in range(TEND + 1):
                sb = (t % 2) * SLOT
                if t <= T - 1:
                    vector.wait_ge(sems["s_act0"], t + 1)
                    vector.tensor_tensor(tmpa_sb[:, :], g0_sb[:, 0:SL], c0_sb[:, :], MUL)
                    vector.tensor_tensor(tmpb_sb[:, :], g0_sb[:, SL:2 * SL], t0_sb[:, :], MUL)
                    vector.tensor_add(c0_sb[:, :], tmpa_sb[:, :], tmpb_sb[:, :]).then_inc(
                        sems["s_cmid0"], 1
                    )
                    vector.wait_ge(sems["s_th0"], t + 1)
                    vector.tensor_tensor(
                        h0l_sb[:, :], g0_sb[:, 2 * SL:3 * SL], th0_sb[:, :], MUL
                    ).then_inc(sems["s_dve0"], 1)
                    vector.wait_ge(sems["s_t0"], t + 1)
                    if t >= 2:
                        vector.wait_ge(sems["s_snt1"], SNT1(t - 2))
                        vector.wait_ge(sems["s_snt2"], SNT2(t - 2))
                    vector.tensor_copy(send_sb[:, sb:sb + 128], psT0[:, 0:128]).then_inc(
                        sems["s_cp0"], 1
                    )
                if 1 <= t <= T:
                    vector.wait_ge(sems["s_act1"], t)
                    vector.tensor_tensor(tmpa_sb[:, :], g1_sb[:, 0:SL], c1_sb[:, :], MUL)
                    vector.tensor_tensor(tmpb_sb[:, :], g1_sb[:, SL:2 * SL], t1_sb[:, :], MUL)
                    vector.tensor_add(c1_sb[:, :], tmpa_sb[:, :], tmpb_sb[:, :]).then_inc(
                        sems["s_cmid1"], 1
                    )
                    vector.wait_ge(sems["s_th1"], t)
                    vector.tensor_tensor(
                        h1l_sb[:, :], g1_sb[:, 2 * SL:3 * SL], th1_sb[:, :], MUL
                    ).then_inc(sems["s_dve1"], 1)
                    vector.wait_ge(sems["s_t1"], t)
                    if t >= 2:
                        vector.wait_ge(sems["s_snt1"], SNT1(t - 2))
                        vector.wait_ge(sems["s_snt2"], SNT2(t - 2))
                    vector.tensor_copy(send_sb[:, sb + 128:sb + 256], psT1[:, 0:128]).then_inc(
                        sems["s_cp1"], 1
                    )
                if 3 <= t <= T + 2:
                    # copy logits(t-3) psum -> lg ring slot (t-3)%32
                    vector.wait_ge(sems["s_lg"], t - 2)
                    q = t - 3
                    if q >= 32:
                        # slot reused: chunk (q//16 - 2) must be stored
                        vector.wait_ge(sems["s_out"], 16 * (q // 16 - 1))
                    vector.tensor_copy(
                        lg_sb[:, (q % 32) * OSL:(q % 32 + 1) * OSL], psL[:, :]
                    ).then_inc(sems["s_lgcp"], 1)

        @block.gpsimd
        def _(gpsimd):
            from concourse import library_config
            gpsimd.load_library(library_config.remote_dma)
            rank = gpsimd.partition_id()
            rreg = gpsimd.to_reg(rank)
            n_prep = 0
            n_out = 0
            for t in range(TEND + 1):
                b1t = has_bc1(t)
                b2t = has_bc2(t)
                sb = (t % 2) * SLOT
                so = slot_off(t)
                # relay: collapse per-sender arrival sems into s_rdy for tensor
                if 1 <= t:
                    if t <= T:  # bc1(t-1) from every sender
                        for k in range(NC):
                            gpsimd.wait_ge(s_a1[k], 2 * t)
                        gpsimd.sem_inc(sems["s_rdy1"], 1)
                    # bc2(t-1) from every sender (exists through t-1 = T+1)
                    for k in range(NC):
                        gpsimd.wait_ge(s_a2[k], 2 * t)
                    gpsimd.sem_inc(sems["s_rdy2"], 1)
                if b1t or b2t:
                    # prep the broadcasts (source is read at trigger time)
                    for k in range(NC):
                        with gpsimd.If_eq(rreg, k):
                            if b1t:
                                gpsimd.remote_dma_broadcast(
                                    hT_sb[:, so + k * SLOT:so + k * SLOT + 128],
                                    send_sb[:, sb:sb + 128],
                                    remote_sem=s_a1[k],
                                    local_sem=sems["s_snt1"],
                                    rdests=[(0, j) for j in range(NC)],
                                ).then_inc(sems["s_prep"], 1)
                            if b2t:
                                gpsimd.remote_dma_broadcast(
                                    hT_sb[:, so + k * SLOT + 128:so + k * SLOT + 384],
                                    send_sb[:, sb + 128:sb + 384],
                                    remote_sem=s_a2[k],
                                    local_sem=sems["s_snt2"],
                                    rdests=[(0, j) for j in range(NC)],
                                ).then_inc(sems["s_prep"], 1)
                    n_prep += int(b1t) + int(b2t)
                    gpsimd.wait_ge(sems["s_prep"], n_prep)
                if b1t:
                    gpsimd.wait_ge(sems["s_cp0"], t + 1)
                    if t >= 1:
                        gpsimd.wait_ge(sems["s_snt1"], SNT1(t - 1))
                    gpsimd.trigger_dma(1)
                if b2t:
                    if t >= 1:
                        gpsimd.wait_ge(sems["s_snt2"], SNT2(t - 1))
                        gpsimd.wait_ge(sems["s_cp1"], min(t, T))
                    if t >= 2:
                        gpsimd.wait_ge(sems["s_hsT"], min(t - 1, T))
                    if t == 0:
                        gpsimd.wait_ge(sems["s_vinit"], 1)  # send buf memset
                    gpsimd.trigger_dma(1)
                # output DMA: store logits every 16 steps (+ final tail)
                q = t - 3
                if 0 <= q <= T - 1 and (q % 16 == 15 or q == T - 1):
                    c = q // 16
                    start = c * 16
                    gpsimd.wait_ge(sems["s_lgcp"], q + 1)
                    if c >= 1:
                        gpsimd.wait_ge(sems["s_out"], 16 * c)
                    half = (c % 2) * 16 * OSL
                    gpsimd.dma_start(
                        out=out[:, start * OSL:(q + 1) * OSL],
                        in_=lg_sb[:, half:half + (q + 1 - start) * OSL],
                    ).then_inc(sems["s_out"], 16)
                    n_out += 1
            gpsimd.wait_ge(sems["s_out"], 16 * n_out)

    # Raw-Bass mode skips the Bacc pipeline, so encode the extended-ISA
    # instructions (remote DMA descs / trigger / library load) here —
    # walrus rejects them unencoded ("ISA wrong length").
    mybir.codegen_inst_isa_subclasses(nc)
    return nc


def _host_prep(inputs, T):
    inp = np.ascontiguousarray(inputs["inputs"][:T]).astype(np.float32)
    emb_W = inputs["emb_W"].astype(np.float32)
    W0 = inputs["lstm_W0"].astype(np.float32)
    b0 = inputs["lstm_b0"].astype(np.float32)
    W1 = inputs["lstm_W1"].astype(np.float32)
    b1 = inputs["lstm_b1"].astype(np.float32)

    flat = inp.reshape(T * B, IND)
    s = np.where(
        (flat[:, VOCAB] == 1.0) & (flat[:, VOCAB + 1] == 0.0), 1.0, -1.0
    ).astype(np.float32)
    inpT_aug = np.zeros((KP, T * B), np.float32)
    inpT_aug[:IND] = flat.T
    inpT_aug[IND] = 1.0
    inpT_aug[IND + 1] = s

    # x-side folded weight: emb @ W0[:512] + flags(b0 row) + rank1(u row)
    Wc = np.zeros((KP, 4 * NN), np.float32)
    Wc[:IND] = emb_W @ W0[:EMB]
    Wc[IND] = b0
    Wc[IND + 1] = W0[EMB:EMB + BIG].sum(axis=0)

    W0h = W0[EMB + BIG:]            # [1024, 4096]
    W1x, W1h = W1[:NN], W1[NN:]

    def gate_cols(W, k):
        return np.concatenate(
            [W[:, base + k * SL:base + (k + 1) * SL] for base in
             (0, NN, 2 * NN, 3 * NN)], axis=1)

    bf = lambda x: np.ascontiguousarray(x).astype(BF16)
    inpT_bf = bf(inpT_aug)
    ow0 = inputs["out_W0"].astype(np.float32)
    ob0 = inputs["out_b0"].astype(np.float32)
    ow1 = inputs["out_W1"].astype(np.float32)
    ob1 = inputs["out_b1"].astype(np.float32)
    iden = bf(np.eye(128, dtype=np.float32))

    in_maps = []
    for k in range(NC):
        in_maps.append({
            "inpT": inpT_bf,
            "wc": bf(gate_cols(Wc, k)),
            "w0h": bf(gate_cols(W0h, k)),
            "w1x": bf(gate_cols(W1x, k)),
            "w1h": bf(gate_cols(W1h, k)),
            "b1r": bf(gate_cols(b1.reshape(1, 4 * NN), k)),
            "ow0k": bf(ow0[:, k * SL:(k + 1) * SL]),
            "ob0k": np.ascontiguousarray(ob0[k * SL:(k + 1) * SL].reshape(128, 1)),
            "ow1k": bf(ow1[:, k * OSL:(k + 1) * OSL]),
            "ob1k": bf(ob1[k * OSL:(k + 1) * OSL].reshape(1, OSL)),
            "iden": iden,
        })
    return in_maps


_CACHE = {}


def run(inputs, T=T_FULL, trace=False):
    if T not in _CACHE:
        _CACHE[T] = build(T)
    nc = _CACHE[T]
    in_maps = _host_prep(inputs, T)
    res = run_bass_kernel_spmd(
        nc, in_maps, core_ids=list(range(NC)), trace=trace
    )
    outs = [
        res.results[k]["out"].reshape(B, T, OSL).transpose(1, 0, 2)
        for k in range(NC)
    ]
    out = np.concatenate(outs, axis=2)
    return out, res


def kernel(**inputs):
    out, _ = run(inputs, T=T_FULL)
    return out.astype(np.float32)
